# revision 50
# baseline (speedup 1.0000x reference)
"""Distributed GQA attention kernel for 8 TRN2 NeuronCores.

Strategy: tensor-parallel over heads, zero collectives.
Each core d holds 4 query heads + 1 kv head (GQA group d). It computes
q/k/v projections (transposed layouts), RoPE, causal attention, and a
partial o_proj (its heads' contribution to every output element). The
host sums the 8 partial outputs (the "unshard" step).

v2 layout decisions (all aimed at keeping the PE busy):
- Attention runs at 128-query granularity with all 4 local heads packed
  side by side, so score/AV/den matmuls stream 512 columns each.
- RoPE's rotate-half is a DVE stream_shuffle (32-lane group swap); the
  head-dim permutation is chosen so each pair's partner sits 16
  partitions away inside the same 32-partition quadrant.
- o_proj PSUM is double-buffered and its evacuations alternate between
  the Vector and Scalar engines.
- x is loaded 4 contraction-chunks per DMA; the o_proj partial output is
  written with one DMA per 128-token row block.
"""
import sys

sys.path.insert(0, '/opt/trn_rl_repo')

import numpy as np
import ml_dtypes

B, T, C = 2, 2048, 4096
H, KVH, HD = 32, 8, 128
NCORES = 8
N = B * T            # 4096 tokens (batches concatenated)
HL = H // NCORES     # 4 local q heads
TB = 256             # token block for projections
NTB = N // TB        # 16
QB = 128             # query block for attention
KB = 128             # key block
NCH = C // 128       # 32 contraction chunks
SCALE = float(1.0 / np.sqrt(HD))

# Head-dim permutation: pair i=(2i,2i+1) lives in quadrant i//16 at
# offsets i%16 (the "a" half) and 16+i%16 (the "b" half), so rotate-half
# becomes a 16<->16 swap inside each 32-partition stream_shuffle group.
PERM = np.empty(128, np.int64)
for _p in range(128):
    _qd, _r = _p // 32, _p % 32
    _i = _qd * 16 + (_r % 16)
    PERM[_p] = 2 * _i + (1 if _r >= 16 else 0)
IMAP = (np.arange(128) // 32) * 16 + (np.arange(128) % 32) % 16
SSIGN = np.where((np.arange(128) % 32) < 16, 1.0, -1.0).astype(np.float32)
SHUF = [(i + 16) % 32 for i in range(32)]

BF16 = ml_dtypes.bfloat16


def _build(dbg=False):
    import concourse.mybir as mybir
    import concourse.tile as tile
    from concourse import bacc

    dt = mybir.dt
    nc = bacc.Bacc("TRN2", target_bir_lowering=False, debug=False)

    xT_d = nc.declare_dram_parameter("xT", [C, N], dt.bfloat16, isOutput=False)
    wqT_d = nc.declare_dram_parameter("wqT", [C, HL * HD], dt.bfloat16, isOutput=False)
    wkT_d = nc.declare_dram_parameter("wkT", [C, HD], dt.bfloat16, isOutput=False)
    wvT_d = nc.declare_dram_parameter("wvT", [C, HD], dt.bfloat16, isOutput=False)
    woT_d = nc.declare_dram_parameter("woT", [HL * HD, C], dt.bfloat16, isOutput=False)
    cosb_d = nc.declare_dram_parameter("cosb", [128, N], dt.bfloat16, isOutput=False)
    sinb_d = nc.declare_dram_parameter("sinb", [128, N], dt.bfloat16, isOutput=False)
    mask_d = nc.declare_dram_parameter("mask", [128, HL * QB], dt.bfloat16, isOutput=False)
    out_d = nc.declare_dram_parameter("out", [N, C], dt.bfloat16, isOutput=True)

    with tile.TileContext(nc) as tc:
        with (
            tc.tile_pool(name="wts", bufs=1) as wts,
            tc.tile_pool(name="cache", bufs=1) as cache,
            tc.tile_pool(name="xin", bufs=16) as xin,
            tc.tile_pool(name="qk", bufs=2) as qkp,
            tc.tile_pool(name="vt", bufs=2) as vtp,
            tc.tile_pool(name="rope", bufs=5) as ropep,
            tc.tile_pool(name="pt", bufs=8) as ptp,
            tc.tile_pool(name="ps", bufs=2) as psp,
            tc.tile_pool(name="att", bufs=2) as attp,
            tc.tile_pool(name="dn", bufs=2) as dnp,
            tc.tile_pool(name="oev", bufs=3) as oevp,
            tc.tile_pool(name="acc", bufs=2, space="PSUM") as accp,
            tc.tile_pool(name="sps", bufs=2, space="PSUM") as spsp,
            tc.tile_pool(name="avp", bufs=1, space="PSUM") as avpp,
            tc.tile_pool(name="dnp", bufs=1, space="PSUM") as dppp,
            tc.tile_pool(name="ops", bufs=2, space="PSUM") as opsp,
        ):
            # ---------------- resident weights / constants ----------------
            wq_s = wts.tile([128, NCH * HL * 128], dt.bfloat16)   # (c,h) -> col (c*HL+h)*128
            wk_s = wts.tile([128, NCH * 128], dt.bfloat16)
            wv_s = wts.tile([128, NCH * 128], dt.bfloat16)
            wo_s = wts.tile([128, HL * C], dt.bfloat16)           # (h,ct) -> col h*C+ct*512
            cos_s = wts.tile([128, N], dt.bfloat16)
            sin_s = wts.tile([128, N], dt.bfloat16)
            mask_s = wts.tile([128, HL * QB], dt.bfloat16)
            ones_s = wts.tile([128, 128], dt.bfloat16)

            nc.any.memset(ones_s[:, :], 1.0)
            # Startup-critical DMA schedule: the first projection matmuls
            # need x(block0, chunk0) + wq[c0] within a few us, so stream
            # weights in consumption order with exponentially growing
            # pieces on the Sync queue, and push cos/sin/wo (needed tens of
            # us later) to the Activation queue.
            wq_v = wq_s[:, :].rearrange("p (c m) -> p c m", c=NCH)
            wqT_v = wqT_d[:, :].rearrange("(c p) m -> p c m", p=128)
            wo_v = wo_s[:, :].rearrange("p (h m) -> p h m", h=HL)
            woT_v = woT_d[:, :].rearrange("(h p) m -> p h m", p=128)
            wk_v = wk_s[:, :].rearrange("p (c m) -> p c m", c=NCH)
            wkT_v = wkT_d[:, :].rearrange("(c p) m -> p c m", p=128)
            wv_v = wv_s[:, :].rearrange("p (c m) -> p c m", c=NCH)
            wvT_v = wvT_d[:, :].rearrange("(c p) m -> p c m", p=128)

            # only the startup-critical halves go out at t=0; the rest is
            # emitted inside BC(0) (see units_bc) so it doesn't steal DMA
            # bandwidth from the first projection blocks
            nc.scalar.dma_start(cos_s[:, 0:512], cosb_d[:, 0:512])
            nc.scalar.dma_start(sin_s[:, 0:512], sinb_d[:, 0:512])
            for i in range(HL):
                nc.scalar.dma_start(wo_v[:, i, 0:2048], woT_v[:, i, 0:2048])

            xT_v = xT_d[:, :].rearrange("(c p) m -> p c m", p=128)
            pre_xcs = []
            for ci in range(16):
                xc = xin.tile([128, 4 * TB], dt.bfloat16, tag="xc", name="xc")
                pre_xcs.append(xc[:, :].rearrange("p (c m) -> p c m", c=4))

            def wq_piece(c0, c1):
                nc.sync.dma_start(wq_v[:, c0:c1], wqT_v[:, c0:c1])

            def xc_piece(ci, j0, j1):
                # block-0/1 x rides the (idle) gpsimd queue so x and weight
                # issuance run in parallel at startup
                tb0 = ci // 8
                nsl0 = slice(tb0 * TB, (tb0 + 1) * TB)
                nc.gpsimd.dma_start(
                    pre_xcs[ci][:, j0:j1],
                    xT_v[:, (ci % 8) * 4 + j0:(ci % 8) * 4 + j1, nsl0])

            xc_piece(0, 0, 1)
            xc_piece(0, 1, 2)
            xc_piece(0, 2, 4)
            xc_piece(1, 0, 2)
            xc_piece(1, 2, 4)
            for ci in range(2, 16):
                xc_piece(ci, 0, 4)
            wq_piece(0, 1)
            nc.sync.dma_start(wk_v[:, 0:2], wkT_v[:, 0:2])
            nc.sync.dma_start(wv_v[:, 0:2], wvT_v[:, 0:2])
            nc.sync.dma_start(mask_s[:, :], mask_d[:, :])
            wq_piece(1, 2)
            wq_piece(2, 4)
            nc.sync.dma_start(wk_v[:, 2:16], wkT_v[:, 2:16])
            wq_piece(4, 8)
            nc.sync.dma_start(wv_v[:, 2:16], wvT_v[:, 2:16])
            wq_piece(8, 12)
            wq_piece(12, 16)
            wq_piece(16, 20)
            nc.sync.dma_start(wk_v[:, 16:32], wkT_v[:, 16:32])
            wq_piece(20, 24)
            nc.sync.dma_start(wv_v[:, 16:32], wvT_v[:, 16:32])
            wq_piece(24, 28)
            wq_piece(28, 32)

            # per-block k/v cache tiles: exact (uncoarsened) dependency
            # tracking so attention reads never falsely wait on later
            # projection writes. kc: [hd, tok]; vc: [tok%128, ti*128+hd].
            kc = [cache.tile([128, TB], dt.bfloat16, name=f"kc{i}")
                  for i in range(NTB)]
            vc = [cache.tile([128, TB], dt.bfloat16, name=f"vc{i}")
                  for i in range(NTB)]

            deferred_vt = []

            def emit_deferred_vt():
                while deferred_vt:
                    vtb, vtmp = deferred_vt.pop(0)
                    for ti in range(2):
                        nc.sync.dma_start_transpose(
                            vc[vtb][:, ti * 128:(ti + 1) * 128],
                            vtmp[:, ti * 128:(ti + 1) * 128])

            def rope_mul(src, nsl):
                # psum readers first so the accumulator bank frees early
                m1 = ropep.tile([128, TB], dt.float32, tag="m1")
                nc.vector.tensor_mul(m1[:, :], src, cos_s[:, nsl])
                u = ropep.tile([128, TB], dt.float32, tag="u")
                nc.vector.tensor_mul(u[:, :], src, sin_s[:, nsl])
                return m1, u

            def rope_fin(dst, m1, u):
                # dst = m1 + shuffle16(u)
                sw = ropep.tile([128, TB], dt.float32, tag="sw")
                nc.vector.stream_shuffle(sw[:, :], u[:, :], SHUF)
                nc.vector.tensor_add(dst, m1[:, :], sw[:, :])

            def units_a(tb, ready_xcs=None):
                """Closure list for the projection phase of block tb.
                Returns (units, q_qv). Units must be called in order."""
                nsl = slice(tb * TB, (tb + 1) * TB)
                st_ = {}

                def u_start():
                    if ready_xcs is not None:
                        st_['xcs'] = ready_xcs
                        if tb > 0:
                            emit_deferred_vt()
                    else:
                        xcs = []
                        for ci in range(8):
                            xc = xin.tile([128, 4 * TB], dt.bfloat16,
                                          tag="xc", name="xc")
                            xc_v = xc[:, :].rearrange("p (c m) -> p c m", c=4)
                            nc.sync.dma_start(
                                xc_v[:, :, :],
                                xT_d[:, nsl].rearrange("(c p) m -> p c m", p=128)[
                                    :, ci * 4:(ci + 1) * 4])
                            xcs.append(xc_v)
                        st_['xcs'] = xcs
                        emit_deferred_vt()
                    st_['t0'] = accp.tile([128, 512], dt.float32, tag="acc",
                                          name="t0")
                    st_['t2'] = accp.tile([128, 512], dt.float32, tag="acc",
                                          name="t2")

                def u_p1(c0):
                    t0, t2, xcs = st_['t0'], st_['t2'], st_['xcs']
                    for c in range(c0, c0 + 2):
                        xc = xcs[c // 4][:, c % 4, :]
                        st = c == 0
                        sp = c == NCH - 1
                        for h in range(2):
                            nc.tensor.matmul(
                                t0[:, h * 256:(h + 1) * 256],
                                wq_s[:, (c * HL + h) * 128:(c * HL + h + 1) * 128],
                                xc, start=st and h == 0, stop=sp)
                        nc.tensor.matmul(
                            t2[:, 0:256],
                            wk_s[:, c * 128:(c + 1) * 128], xc,
                            start=st, stop=sp)
                        # v in k-style ([hd, tok]); transposed to cache
                        # layout later via DMA transpose
                        nc.tensor.matmul(
                            t2[:, 256:512],
                            wv_s[:, c * 128:(c + 1) * 128], xc,
                            start=False, stop=sp)

                def u_rope1():
                    t0, t2 = st_['t0'], st_['t2']
                    mq0 = rope_mul(t0[:, 0:256], nsl)
                    mq1 = rope_mul(t0[:, 256:512], nsl)      # frees t0
                    mk = rope_mul(t2[:, 0:256], nsl)
                    vtmp = vtp.tile([128, TB], dt.bfloat16, tag="vtmp",
                                    name="vtmp")
                    nc.vector.tensor_copy(vtmp[:, :], t2[:, 256:512])
                    # transposes into the v cache are DEFERRED: emitted on
                    # the Sync queue only after the next block's x-prefetch
                    # DMAs, by which time vtmp is ready (no queue blocking)
                    deferred_vt.append((tb, vtmp))
                    rope_fin(q_qv[:, 0, :], *mq0)
                    rope_fin(q_qv[:, 1, :], *mq1)
                    rope_fin(kc[tb][:, :], *mk)
                    st_['t1'] = accp.tile([128, 512], dt.float32, tag="acc",
                                          name="t1")

                def u_p2(c0):
                    t1, xcs = st_['t1'], st_['xcs']
                    for c in range(c0, c0 + 2):
                        xc = xcs[c // 4][:, c % 4, :]
                        sp = c == NCH - 1
                        for h in range(2):
                            nc.tensor.matmul(
                                t1[:, h * 256:(h + 1) * 256],
                                wq_s[:, (c * HL + h + 2) * 128:(c * HL + h + 3) * 128],
                                xc, start=c == 0 and h == 0, stop=sp)

                def u_rope2():
                    t1 = st_['t1']
                    mq2 = rope_mul(t1[:, 0:256], nsl)
                    mq3 = rope_mul(t1[:, 256:512], nsl)      # frees t1
                    rope_fin(q_qv[:, 2, :], *mq2)
                    rope_fin(q_qv[:, 3, :], *mq3)

                q_sb = qkp.tile([128, HL * TB], dt.bfloat16, tag="qsb",
                                name="q_sb")
                q_qv = q_sb[:, :].rearrange("p (h q) -> p h q", h=HL)
                units = [u_start]
                units += [(lambda c0=c0: u_p1(c0)) for c0 in range(0, NCH, 2)]
                units.append(u_rope1)
                units += [(lambda c0=c0: u_p2(c0)) for c0 in range(0, NCH, 2)]
                units.append(u_rope2)
                return units, q_qv

            def units_bc(tb, q_qv):
                """Closure list for attention + o_proj of block tb."""
                b = tb // 8
                units = []
                st_ = {}
                for qh in range(2):
                    qbl = (tb % 8) * 2 + qh        # in-batch 128-query block
                    nkb = qbl + 1
                    qrhs = q_qv[:, :, qh * 128:(qh + 1) * 128]

                    def u_binit(qh=qh):
                        if tb == 0 and qh == 0:
                            # deferred bulk loads: needed from ~40us on
                            for i in range(HL):
                                nc.sync.dma_start(wo_v[:, i, 2048:C],
                                                  woT_v[:, i, 2048:C])
                            nc.sync.dma_start(cos_s[:, 512:N],
                                              cosb_d[:, 512:N])
                            nc.sync.dma_start(sin_s[:, 512:N],
                                              sinb_d[:, 512:N])
                        st_[qh, 'at4'] = avpp.tile([128, 512], dt.float32,
                                                   tag="at4", name="at4")
                        st_[qh, 'den'] = dppp.tile([128, 512], dt.float32,
                                                   tag="den", name="den")
                    units.append(u_binit)

                    def u_kbl(kbl, qh=qh, nkb=nkb, qrhs=qrhs):
                        at4, den = st_[qh, 'at4'], st_[qh, 'den']
                        kbg = b * 16 + kbl
                        sT = spsp.tile([128, 512], dt.float32, tag="sT",
                                       name="sT")
                        nc.tensor.matmul(
                            sT[:, :], kc[kbg // 2][:, (kbg % 2) * 128:(kbg % 2 + 1) * 128],
                            qrhs, start=True, stop=True)
                        pT = ptp.tile([128, 512], dt.bfloat16, tag="pT",
                                      name="pT")
                        nc.scalar.activation(pT[:, :], sT[:, :],
                                             mybir.ActivationFunctionType.Exp,
                                             scale=SCALE)
                        if kbl == nkb - 1:
                            nc.vector.tensor_mul(pT[:, :], pT[:, :], mask_s[:, :])
                        nc.tensor.matmul(
                            at4[:, :],
                            vc[kbg // 2][:, (kbg % 2) * 128:(kbg % 2 + 1) * 128],
                            pT[:, :], start=kbl == 0, stop=kbl == nkb - 1)
                        # denominator: fold up to 4 pT tiles with cheap bf16
                        # DVE adds so only every 4th kbl pays a ones-matmul
                        grp = st_.setdefault((qh, 'grp'), [])
                        grp.append(pT)
                        if len(grp) == 4 or kbl == nkb - 1:
                            acc = grp[0]
                            for nxt in grp[1:]:
                                ps = psp.tile([128, 512], dt.bfloat16,
                                              tag="ps", name="ps")
                                nc.vector.tensor_add(ps[:, :], acc[:, :],
                                                     nxt[:, :])
                                acc = ps
                            nc.tensor.matmul(den[:, :], ones_s[:, :],
                                             acc[:, :], start=kbl < 4,
                                             stop=kbl == nkb - 1)
                            grp.clear()
                    units += [(lambda kbl=kbl, f=u_kbl: f(kbl))
                              for kbl in range(nkb)]

                    def u_soft(qh=qh):
                        at4, den = st_[qh, 'at4'], st_[qh, 'den']
                        denb = dnp.tile([128, 512], dt.float32, tag="denb",
                                        name="denb")
                        nc.vector.reciprocal_approx_fast(denb[:, :], den[:, :])
                        attn = attp.tile([128, 512], dt.bfloat16, tag="attn",
                                         name="attn")
                        nc.vector.tensor_mul(attn[:, :], at4[:, :], denb[:, :])
                        st_[qh, 'attn'] = attn
                        st_[qh, 'oev'] = oevp.tile([128, C], dt.bfloat16,
                                                   tag="oev", name="oev")
                    units.append(u_soft)

                    def u_oproj(ct, qh=qh):
                        attn, oev = st_[qh, 'attn'], st_[qh, 'oev']
                        ops = opsp.tile([128, 512], dt.float32, tag="ops",
                                        name="ops")
                        for h in range(HL):
                            nc.tensor.matmul(
                                ops[:, :],
                                attn[:, h * 128:(h + 1) * 128],
                                wo_s[:, h * C + ct * 512:h * C + (ct + 1) * 512],
                                start=(h == 0), stop=(h == HL - 1))
                        osl = slice(ct * 512, (ct + 1) * 512)
                        if ct % 2 == 0:
                            nc.vector.tensor_copy(oev[:, osl], ops[:, :])
                        else:
                            nc.scalar.activation(
                                oev[:, osl], ops[:, :],
                                mybir.ActivationFunctionType.Copy)
                        if tb == NTB - 1 and ct % 2 == 1:
                            # last block: drain the output in quarters so
                            # the final DMA isn't serialized after compute
                            r0 = tb * TB + qh * 128
                            nc.sync.dma_start(
                                out_d[r0:r0 + 128, (ct - 1) * 512:(ct + 1) * 512],
                                oev[:, (ct - 1) * 512:(ct + 1) * 512])
                    units += [(lambda ct=ct, f=u_oproj: f(ct))
                              for ct in range(C // 512)]

                    def u_dma(qh=qh):
                        if tb == NTB - 1:
                            return
                        r0 = tb * TB + qh * 128
                        nc.gpsimd.dma_start(out_d[r0:r0 + 128, :],
                                            st_[qh, 'oev'][:, :])
                    units.append(u_dma)
                return units

            def run_merged(ua, ub):
                # proportional stable merge of the two unit lists
                seq = [((i + 0.5) / len(ua), 0, i, u) for i, u in enumerate(ua)]
                seq += [((i + 0.5) / len(ub), 1, i, u) for i, u in enumerate(ub)]
                for _, _, _, u in sorted(seq, key=lambda t: (t[0], t[1])):
                    u()

            # software pipeline: interleave the emission of A(tb) with
            # B/C(tb-1) so both phases' work is pending at every point and
            # the scheduler can fill any cross-engine stall.
            prev = None
            for tb in range(NTB):
                rx = pre_xcs[8 * tb:8 * tb + 8] if tb < 2 else None
                ua, q_qv = units_a(tb, ready_xcs=rx)
                if prev is None:
                    for u in ua:
                        u()
                else:
                    run_merged(ua, units_bc(tb - 1, prev))
                prev = q_qv
            emit_deferred_vt()
            for u in units_bc(NTB - 1, prev):
                u()
    nc.finalize()
    return nc


def _prep_shared(x, freqs_cis):
    xf = np.asarray(x, np.float32).reshape(N, C)
    xT = np.ascontiguousarray(xf.T).astype(BF16)
    fc = np.asarray(freqs_cis, np.float32)
    cos = np.ascontiguousarray(fc[:, :, 0].T)   # [64, T]
    sin = np.ascontiguousarray(fc[:, :, 1].T)
    cosb = np.ascontiguousarray(np.tile(cos[IMAP], (1, B))).astype(BF16)
    sinb = np.ascontiguousarray(
        np.tile(sin[IMAP] * SSIGN[:, None], (1, B))).astype(BF16)
    # diagonal-block causal mask [key, (head, query)]: key <= query
    m = np.triu(np.ones((KB, QB), np.float32)).astype(BF16)
    mask = np.ascontiguousarray(np.tile(m, (1, HL)))
    return xT, cosb, sinb, mask


def _prep_core(d, wq_p, wk_p, wv_f, wo_f):
    qsl = slice(d * HL * HD, (d + 1) * HL * HD)
    ksl = slice(d * HD, (d + 1) * HD)
    wqT = np.ascontiguousarray(wq_p[qsl].T).astype(BF16)
    wkT = np.ascontiguousarray(wk_p[ksl].T).astype(BF16)
    wvT = np.ascontiguousarray(wv_f[ksl].T).astype(BF16)
    woT = np.ascontiguousarray(wo_f[:, qsl].T).astype(BF16)
    return wqT, wkT, wvT, woT


_NC_CACHE = []


def kernel(x, freqs_cis, wq, wk, wv, wo):
    from concourse import bass_utils

    if not _NC_CACHE:
        _NC_CACHE.append(_build())
    nc = _NC_CACHE[0]

    xT, cosb, sinb, mask = _prep_shared(x, freqs_cis)
    wq_p = np.asarray(wq, np.float32).reshape(H, HD, C)[:, PERM, :].reshape(H * HD, C)
    wk_p = np.asarray(wk, np.float32).reshape(KVH, HD, C)[:, PERM, :].reshape(KVH * HD, C)
    wv_f = np.asarray(wv, np.float32)
    wo_f = np.asarray(wo, np.float32)

    in_maps = []
    for d in range(NCORES):
        wqT, wkT, wvT, woT = _prep_core(d, wq_p, wk_p, wv_f, wo_f)
        in_maps.append({
            "xT": xT, "wqT": wqT, "wkT": wkT, "wvT": wvT, "woT": woT,
            "cosb": cosb, "sinb": sinb, "mask": mask,
        })
    res = bass_utils.run_bass_kernel_spmd(nc, in_maps, core_ids=list(range(NCORES)))
    acc = np.zeros((N, C), np.float32)
    for r in res.results:
        acc += np.asarray(r["out"], np.float32)
    return acc.reshape(B, T, C)


# revision 53
# speedup vs baseline: 1.0040x; 1.0040x over previous
"""Distributed GQA attention kernel for 8 TRN2 NeuronCores.

Strategy: tensor-parallel over heads, zero collectives.
Each core d holds 4 query heads + 1 kv head (GQA group d). It computes
q/k/v projections (transposed layouts), RoPE, causal attention, and a
partial o_proj (its heads' contribution to every output element). The
host sums the 8 partial outputs (the "unshard" step).

v2 layout decisions (all aimed at keeping the PE busy):
- Attention runs at 128-query granularity with all 4 local heads packed
  side by side, so score/AV/den matmuls stream 512 columns each.
- RoPE's rotate-half is a DVE stream_shuffle (32-lane group swap); the
  head-dim permutation is chosen so each pair's partner sits 16
  partitions away inside the same 32-partition quadrant.
- o_proj PSUM is double-buffered and its evacuations alternate between
  the Vector and Scalar engines.
- x is loaded 4 contraction-chunks per DMA; the o_proj partial output is
  written with one DMA per 128-token row block.
"""
import sys

sys.path.insert(0, '/opt/trn_rl_repo')

import numpy as np
import ml_dtypes

B, T, C = 2, 2048, 4096
H, KVH, HD = 32, 8, 128
NCORES = 8
N = B * T            # 4096 tokens (batches concatenated)
HL = H // NCORES     # 4 local q heads
TB = 256             # token block for projections
NTB = N // TB        # 16
QB = 128             # query block for attention
KB = 128             # key block
NCH = C // 128       # 32 contraction chunks
SCALE = float(1.0 / np.sqrt(HD))

# Head-dim permutation: pair i=(2i,2i+1) lives in quadrant i//16 at
# offsets i%16 (the "a" half) and 16+i%16 (the "b" half), so rotate-half
# becomes a 16<->16 swap inside each 32-partition stream_shuffle group.
PERM = np.empty(128, np.int64)
for _p in range(128):
    _qd, _r = _p // 32, _p % 32
    _i = _qd * 16 + (_r % 16)
    PERM[_p] = 2 * _i + (1 if _r >= 16 else 0)
IMAP = (np.arange(128) // 32) * 16 + (np.arange(128) % 32) % 16
SSIGN = np.where((np.arange(128) % 32) < 16, 1.0, -1.0).astype(np.float32)
SHUF = [(i + 16) % 32 for i in range(32)]

BF16 = ml_dtypes.bfloat16


def _build(dbg=False):
    import concourse.mybir as mybir
    import concourse.tile as tile
    from concourse import bacc

    dt = mybir.dt
    nc = bacc.Bacc("TRN2", target_bir_lowering=False, debug=False)

    xT_d = nc.declare_dram_parameter("xT", [C, N], dt.bfloat16, isOutput=False)
    wqT_d = nc.declare_dram_parameter("wqT", [C, HL * HD], dt.bfloat16, isOutput=False)
    wkT_d = nc.declare_dram_parameter("wkT", [C, HD], dt.bfloat16, isOutput=False)
    wvT_d = nc.declare_dram_parameter("wvT", [C, HD], dt.bfloat16, isOutput=False)
    woT_d = nc.declare_dram_parameter("woT", [HL * HD, C], dt.bfloat16, isOutput=False)
    cosb_d = nc.declare_dram_parameter("cosb", [128, N], dt.bfloat16, isOutput=False)
    sinb_d = nc.declare_dram_parameter("sinb", [128, N], dt.bfloat16, isOutput=False)
    mask_d = nc.declare_dram_parameter("mask", [128, HL * QB], dt.bfloat16, isOutput=False)
    out_d = nc.declare_dram_parameter("out", [N, C], dt.bfloat16, isOutput=True)

    with tile.TileContext(nc) as tc:
        with (
            tc.tile_pool(name="wts", bufs=1) as wts,
            tc.tile_pool(name="cache", bufs=1) as cache,
            tc.tile_pool(name="xin", bufs=16) as xin,
            tc.tile_pool(name="qk", bufs=2) as qkp,
            tc.tile_pool(name="vt", bufs=2) as vtp,
            tc.tile_pool(name="rope", bufs=5) as ropep,
            tc.tile_pool(name="pt", bufs=8) as ptp,
            tc.tile_pool(name="ps", bufs=2) as psp,
            tc.tile_pool(name="att", bufs=2) as attp,
            tc.tile_pool(name="dn", bufs=2) as dnp,
            tc.tile_pool(name="oev", bufs=3) as oevp,
            tc.tile_pool(name="acc", bufs=2, space="PSUM") as accp,
            tc.tile_pool(name="sps", bufs=2, space="PSUM") as spsp,
            tc.tile_pool(name="avp", bufs=1, space="PSUM") as avpp,
            tc.tile_pool(name="dnp", bufs=1, space="PSUM") as dppp,
            tc.tile_pool(name="ops", bufs=2, space="PSUM") as opsp,
        ):
            # ---------------- resident weights / constants ----------------
            wq_s = wts.tile([128, NCH * HL * 128], dt.bfloat16)   # (c,h) -> col (c*HL+h)*128
            wk_s = wts.tile([128, NCH * 128], dt.bfloat16)
            wv_s = wts.tile([128, NCH * 128], dt.bfloat16)
            wo_s = wts.tile([128, HL * C], dt.bfloat16)           # (h,ct) -> col h*C+ct*512
            cos_s = wts.tile([128, N], dt.bfloat16)
            sin_s = wts.tile([128, N], dt.bfloat16)
            mask_s = wts.tile([128, HL * QB], dt.bfloat16)
            ones_s = wts.tile([128, 128], dt.bfloat16)

            nc.any.memset(ones_s[:, :], 1.0)
            # Startup-critical DMA schedule: the first projection matmuls
            # need x(block0, chunk0) + wq[c0] within a few us, so stream
            # weights in consumption order with exponentially growing
            # pieces on the Sync queue, and push cos/sin/wo (needed tens of
            # us later) to the Activation queue.
            wq_v = wq_s[:, :].rearrange("p (c m) -> p c m", c=NCH)
            wqT_v = wqT_d[:, :].rearrange("(c p) m -> p c m", p=128)
            wo_v = wo_s[:, :].rearrange("p (h m) -> p h m", h=HL)
            woT_v = woT_d[:, :].rearrange("(h p) m -> p h m", p=128)
            wk_v = wk_s[:, :].rearrange("p (c m) -> p c m", c=NCH)
            wkT_v = wkT_d[:, :].rearrange("(c p) m -> p c m", p=128)
            wv_v = wv_s[:, :].rearrange("p (c m) -> p c m", c=NCH)
            wvT_v = wvT_d[:, :].rearrange("(c p) m -> p c m", p=128)

            # only the startup-critical halves go out at t=0; the rest is
            # emitted inside BC(0) (see units_bc) so it doesn't steal DMA
            # bandwidth from the first projection blocks
            nc.scalar.dma_start(cos_s[:, 0:512], cosb_d[:, 0:512])
            nc.scalar.dma_start(sin_s[:, 0:512], sinb_d[:, 0:512])
            for i in range(HL):
                nc.scalar.dma_start(wo_v[:, i, 0:2048], woT_v[:, i, 0:2048])

            xT_v = xT_d[:, :].rearrange("(c p) m -> p c m", p=128)
            pre_xcs = []
            for ci in range(16):
                xc = xin.tile([128, 4 * TB], dt.bfloat16, tag="xc", name="xc")
                pre_xcs.append(xc[:, :].rearrange("p (c m) -> p c m", c=4))

            def wq_piece(c0, c1):
                nc.sync.dma_start(wq_v[:, c0:c1], wqT_v[:, c0:c1])

            def xc_piece(ci, j0, j1):
                # block-0/1 x rides the (idle) gpsimd queue so x and weight
                # issuance run in parallel at startup
                tb0 = ci // 8
                nsl0 = slice(tb0 * TB, (tb0 + 1) * TB)
                nc.gpsimd.dma_start(
                    pre_xcs[ci][:, j0:j1],
                    xT_v[:, (ci % 8) * 4 + j0:(ci % 8) * 4 + j1, nsl0])

            xc_piece(0, 0, 1)
            xc_piece(0, 1, 2)
            xc_piece(0, 2, 4)
            xc_piece(1, 0, 2)
            xc_piece(1, 2, 4)
            for ci in range(2, 16):
                xc_piece(ci, 0, 4)
            wq_piece(0, 1)
            nc.sync.dma_start(wk_v[:, 0:2], wkT_v[:, 0:2])
            nc.sync.dma_start(wv_v[:, 0:2], wvT_v[:, 0:2])
            nc.sync.dma_start(mask_s[:, :], mask_d[:, :])
            wq_piece(1, 2)
            wq_piece(2, 4)
            nc.sync.dma_start(wk_v[:, 2:16], wkT_v[:, 2:16])
            wq_piece(4, 8)
            nc.sync.dma_start(wv_v[:, 2:16], wvT_v[:, 2:16])
            wq_piece(8, 12)
            wq_piece(12, 16)
            wq_piece(16, 20)
            nc.sync.dma_start(wk_v[:, 16:32], wkT_v[:, 16:32])
            wq_piece(20, 24)
            nc.sync.dma_start(wv_v[:, 16:32], wvT_v[:, 16:32])
            wq_piece(24, 28)
            wq_piece(28, 32)

            # per-block k/v cache tiles: exact (uncoarsened) dependency
            # tracking so attention reads never falsely wait on later
            # projection writes. kc: [hd, tok]; vc: [tok%128, ti*128+hd].
            kc = [cache.tile([128, TB], dt.bfloat16, name=f"kc{i}")
                  for i in range(NTB)]
            vc = [cache.tile([128, TB], dt.bfloat16, name=f"vc{i}")
                  for i in range(NTB)]

            deferred_vt = []

            def emit_deferred_vt():
                while deferred_vt:
                    vtb, vtmp = deferred_vt.pop(0)
                    for ti in range(2):
                        nc.sync.dma_start_transpose(
                            vc[vtb][:, ti * 128:(ti + 1) * 128],
                            vtmp[:, ti * 128:(ti + 1) * 128])

            def rope_mul(src, nsl):
                # psum readers first so the accumulator bank frees early
                m1 = ropep.tile([128, TB], dt.float32, tag="m1")
                nc.vector.tensor_mul(m1[:, :], src, cos_s[:, nsl])
                u = ropep.tile([128, TB], dt.float32, tag="u")
                nc.vector.tensor_mul(u[:, :], src, sin_s[:, nsl])
                return m1, u

            def rope_fin(dst, m1, u):
                # dst = m1 + shuffle16(u)
                sw = ropep.tile([128, TB], dt.float32, tag="sw")
                nc.vector.stream_shuffle(sw[:, :], u[:, :], SHUF)
                nc.vector.tensor_add(dst, m1[:, :], sw[:, :])

            def units_a(tb, ready_xcs=None):
                """Closure list for the projection phase of block tb.
                Returns (units, q_qv). Units must be called in order."""
                nsl = slice(tb * TB, (tb + 1) * TB)
                st_ = {}

                def u_start():
                    if ready_xcs is not None:
                        st_['xcs'] = ready_xcs
                        if tb > 0:
                            emit_deferred_vt()
                    else:
                        xcs = []
                        for ci in range(8):
                            xc = xin.tile([128, 4 * TB], dt.bfloat16,
                                          tag="xc", name="xc")
                            xc_v = xc[:, :].rearrange("p (c m) -> p c m", c=4)
                            nc.sync.dma_start(
                                xc_v[:, :, :],
                                xT_d[:, nsl].rearrange("(c p) m -> p c m", p=128)[
                                    :, ci * 4:(ci + 1) * 4])
                            xcs.append(xc_v)
                        st_['xcs'] = xcs
                        emit_deferred_vt()
                    st_['t0'] = accp.tile([128, 512], dt.float32, tag="acc",
                                          name="t0")
                    st_['t2'] = accp.tile([128, 512], dt.float32, tag="acc",
                                          name="t2")

                def u_p1(c0):
                    t0, t2, xcs = st_['t0'], st_['t2'], st_['xcs']
                    for c in range(c0, c0 + 2):
                        xc = xcs[c // 4][:, c % 4, :]
                        st = c == 0
                        sp = c == NCH - 1
                        for h in range(2):
                            nc.tensor.matmul(
                                t0[:, h * 256:(h + 1) * 256],
                                wq_s[:, (c * HL + h) * 128:(c * HL + h + 1) * 128],
                                xc, start=st and h == 0, stop=sp)
                        nc.tensor.matmul(
                            t2[:, 0:256],
                            wk_s[:, c * 128:(c + 1) * 128], xc,
                            start=st, stop=sp)
                        # v in k-style ([hd, tok]); transposed to cache
                        # layout later via DMA transpose
                        nc.tensor.matmul(
                            t2[:, 256:512],
                            wv_s[:, c * 128:(c + 1) * 128], xc,
                            start=False, stop=sp)

                def u_rope1():
                    t0, t2 = st_['t0'], st_['t2']
                    mq0 = rope_mul(t0[:, 0:256], nsl)
                    mq1 = rope_mul(t0[:, 256:512], nsl)      # frees t0
                    mk = rope_mul(t2[:, 0:256], nsl)
                    vtmp = vtp.tile([128, TB], dt.bfloat16, tag="vtmp",
                                    name="vtmp")
                    nc.vector.tensor_copy(vtmp[:, :], t2[:, 256:512])
                    # transposes into the v cache are DEFERRED: emitted on
                    # the Sync queue only after the next block's x-prefetch
                    # DMAs, by which time vtmp is ready (no queue blocking)
                    deferred_vt.append((tb, vtmp))
                    rope_fin(q_qv[:, 0, :], *mq0)
                    rope_fin(q_qv[:, 1, :], *mq1)
                    rope_fin(kc[tb][:, :], *mk)
                    st_['t1'] = accp.tile([128, 512], dt.float32, tag="acc",
                                          name="t1")

                def u_p2(c0):
                    t1, xcs = st_['t1'], st_['xcs']
                    for c in range(c0, c0 + 2):
                        xc = xcs[c // 4][:, c % 4, :]
                        sp = c == NCH - 1
                        for h in range(2):
                            nc.tensor.matmul(
                                t1[:, h * 256:(h + 1) * 256],
                                wq_s[:, (c * HL + h + 2) * 128:(c * HL + h + 3) * 128],
                                xc, start=c == 0 and h == 0, stop=sp)

                def u_rope2():
                    t1 = st_['t1']
                    mq2 = rope_mul(t1[:, 0:256], nsl)
                    mq3 = rope_mul(t1[:, 256:512], nsl)      # frees t1
                    rope_fin(q_qv[:, 2, :], *mq2)
                    rope_fin(q_qv[:, 3, :], *mq3)

                q_sb = qkp.tile([128, HL * TB], dt.bfloat16, tag="qsb",
                                name="q_sb")
                q_qv = q_sb[:, :].rearrange("p (h q) -> p h q", h=HL)
                units = [u_start]
                units += [(lambda c0=c0: u_p1(c0)) for c0 in range(0, NCH, 2)]
                units.append(u_rope1)
                units += [(lambda c0=c0: u_p2(c0)) for c0 in range(0, NCH, 2)]
                units.append(u_rope2)
                return units, q_qv

            def units_bc(tb, q_qv):
                """Closure list for attention + o_proj of block tb."""
                b = tb // 8
                units = []
                st_ = {}
                for qh in range(2):
                    qbl = (tb % 8) * 2 + qh        # in-batch 128-query block
                    nkb = qbl + 1
                    qrhs = q_qv[:, :, qh * 128:(qh + 1) * 128]

                    def u_binit(qh=qh):
                        if tb == 0 and qh == 0:
                            # deferred bulk loads: needed from ~45us on
                            for i in range(HL):
                                nc.sync.dma_start(wo_v[:, i, 2048:C],
                                                  woT_v[:, i, 2048:C])
                        if tb == 1 and qh == 0:
                            # cos/sin bulk: first consumer is RoPE(tb2)
                            nc.sync.dma_start(cos_s[:, 512:N],
                                              cosb_d[:, 512:N])
                            nc.sync.dma_start(sin_s[:, 512:N],
                                              sinb_d[:, 512:N])
                        st_[qh, 'at4'] = avpp.tile([128, 512], dt.float32,
                                                   tag="at4", name="at4")
                        st_[qh, 'den'] = dppp.tile([128, 512], dt.float32,
                                                   tag="den", name="den")
                    units.append(u_binit)

                    def u_kbl(kbl, qh=qh, nkb=nkb, qrhs=qrhs):
                        at4, den = st_[qh, 'at4'], st_[qh, 'den']
                        kbg = b * 16 + kbl
                        sT = spsp.tile([128, 512], dt.float32, tag="sT",
                                       name="sT")
                        nc.tensor.matmul(
                            sT[:, :], kc[kbg // 2][:, (kbg % 2) * 128:(kbg % 2 + 1) * 128],
                            qrhs, start=True, stop=True)
                        pT = ptp.tile([128, 512], dt.bfloat16, tag="pT",
                                      name="pT")
                        nc.scalar.activation(pT[:, :], sT[:, :],
                                             mybir.ActivationFunctionType.Exp,
                                             scale=SCALE)
                        if kbl == nkb - 1:
                            nc.vector.tensor_mul(pT[:, :], pT[:, :], mask_s[:, :])
                        nc.tensor.matmul(
                            at4[:, :],
                            vc[kbg // 2][:, (kbg % 2) * 128:(kbg % 2 + 1) * 128],
                            pT[:, :], start=kbl == 0, stop=kbl == nkb - 1)
                        # denominator: fold up to 4 pT tiles with cheap bf16
                        # DVE adds so only every 4th kbl pays a ones-matmul
                        grp = st_.setdefault((qh, 'grp'), [])
                        grp.append(pT)
                        if len(grp) == 4 or kbl == nkb - 1:
                            acc = grp[0]
                            for nxt in grp[1:]:
                                ps = psp.tile([128, 512], dt.bfloat16,
                                              tag="ps", name="ps")
                                nc.vector.tensor_add(ps[:, :], acc[:, :],
                                                     nxt[:, :])
                                acc = ps
                            nc.tensor.matmul(den[:, :], ones_s[:, :],
                                             acc[:, :], start=kbl < 4,
                                             stop=kbl == nkb - 1)
                            grp.clear()
                    units += [(lambda kbl=kbl, f=u_kbl: f(kbl))
                              for kbl in range(nkb)]

                    def u_soft(qh=qh):
                        at4, den = st_[qh, 'at4'], st_[qh, 'den']
                        denb = dnp.tile([128, 512], dt.float32, tag="denb",
                                        name="denb")
                        nc.vector.reciprocal_approx_fast(denb[:, :], den[:, :])
                        attn = attp.tile([128, 512], dt.bfloat16, tag="attn",
                                         name="attn")
                        nc.vector.tensor_mul(attn[:, :], at4[:, :], denb[:, :])
                        st_[qh, 'attn'] = attn
                        st_[qh, 'oev'] = oevp.tile([128, C], dt.bfloat16,
                                                   tag="oev", name="oev")
                    units.append(u_soft)

                    def u_oproj(ct, qh=qh):
                        attn, oev = st_[qh, 'attn'], st_[qh, 'oev']
                        ops = opsp.tile([128, 512], dt.float32, tag="ops",
                                        name="ops")
                        for h in range(HL):
                            nc.tensor.matmul(
                                ops[:, :],
                                attn[:, h * 128:(h + 1) * 128],
                                wo_s[:, h * C + ct * 512:h * C + (ct + 1) * 512],
                                start=(h == 0), stop=(h == HL - 1))
                        osl = slice(ct * 512, (ct + 1) * 512)
                        if ct % 2 == 0:
                            nc.vector.tensor_copy(oev[:, osl], ops[:, :])
                        else:
                            nc.scalar.activation(
                                oev[:, osl], ops[:, :],
                                mybir.ActivationFunctionType.Copy)
                        if tb >= NTB - 2 and ct % 2 == 1:
                            # last blocks: drain the output in quarters so
                            # the final DMA isn't serialized after compute
                            r0 = tb * TB + qh * 128
                            nc.sync.dma_start(
                                out_d[r0:r0 + 128, (ct - 1) * 512:(ct + 1) * 512],
                                oev[:, (ct - 1) * 512:(ct + 1) * 512])
                    units += [(lambda ct=ct, f=u_oproj: f(ct))
                              for ct in range(C // 512)]

                    def u_dma(qh=qh):
                        if tb >= NTB - 2:
                            return
                        r0 = tb * TB + qh * 128
                        nc.gpsimd.dma_start(out_d[r0:r0 + 128, :],
                                            st_[qh, 'oev'][:, :])
                    units.append(u_dma)
                return units

            def run_merged(ua, ub):
                # proportional stable merge of the two unit lists
                seq = [((i + 0.5) / len(ua), 0, i, u) for i, u in enumerate(ua)]
                seq += [((i + 0.5) / len(ub), 1, i, u) for i, u in enumerate(ub)]
                for _, _, _, u in sorted(seq, key=lambda t: (t[0], t[1])):
                    u()

            # software pipeline: interleave the emission of A(tb) with
            # B/C(tb-1) so both phases' work is pending at every point and
            # the scheduler can fill any cross-engine stall.
            prev = None
            for tb in range(NTB):
                rx = pre_xcs[8 * tb:8 * tb + 8] if tb < 2 else None
                ua, q_qv = units_a(tb, ready_xcs=rx)
                if prev is None:
                    for u in ua:
                        u()
                else:
                    run_merged(ua, units_bc(tb - 1, prev))
                prev = q_qv
            emit_deferred_vt()
            for u in units_bc(NTB - 1, prev):
                u()
    nc.finalize()
    return nc


def _prep_shared(x, freqs_cis):
    xf = np.asarray(x, np.float32).reshape(N, C)
    xT = np.ascontiguousarray(xf.T).astype(BF16)
    fc = np.asarray(freqs_cis, np.float32)
    cos = np.ascontiguousarray(fc[:, :, 0].T)   # [64, T]
    sin = np.ascontiguousarray(fc[:, :, 1].T)
    cosb = np.ascontiguousarray(np.tile(cos[IMAP], (1, B))).astype(BF16)
    sinb = np.ascontiguousarray(
        np.tile(sin[IMAP] * SSIGN[:, None], (1, B))).astype(BF16)
    # diagonal-block causal mask [key, (head, query)]: key <= query
    m = np.triu(np.ones((KB, QB), np.float32)).astype(BF16)
    mask = np.ascontiguousarray(np.tile(m, (1, HL)))
    return xT, cosb, sinb, mask


def _prep_core(d, wq_p, wk_p, wv_f, wo_f):
    qsl = slice(d * HL * HD, (d + 1) * HL * HD)
    ksl = slice(d * HD, (d + 1) * HD)
    wqT = np.ascontiguousarray(wq_p[qsl].T).astype(BF16)
    wkT = np.ascontiguousarray(wk_p[ksl].T).astype(BF16)
    wvT = np.ascontiguousarray(wv_f[ksl].T).astype(BF16)
    woT = np.ascontiguousarray(wo_f[:, qsl].T).astype(BF16)
    return wqT, wkT, wvT, woT


_NC_CACHE = []


def kernel(x, freqs_cis, wq, wk, wv, wo):
    from concourse import bass_utils

    if not _NC_CACHE:
        _NC_CACHE.append(_build())
    nc = _NC_CACHE[0]

    xT, cosb, sinb, mask = _prep_shared(x, freqs_cis)
    wq_p = np.asarray(wq, np.float32).reshape(H, HD, C)[:, PERM, :].reshape(H * HD, C)
    wk_p = np.asarray(wk, np.float32).reshape(KVH, HD, C)[:, PERM, :].reshape(KVH * HD, C)
    wv_f = np.asarray(wv, np.float32)
    wo_f = np.asarray(wo, np.float32)

    in_maps = []
    for d in range(NCORES):
        wqT, wkT, wvT, woT = _prep_core(d, wq_p, wk_p, wv_f, wo_f)
        in_maps.append({
            "xT": xT, "wqT": wqT, "wkT": wkT, "wvT": wvT, "woT": woT,
            "cosb": cosb, "sinb": sinb, "mask": mask,
        })
    res = bass_utils.run_bass_kernel_spmd(nc, in_maps, core_ids=list(range(NCORES)))
    acc = np.zeros((N, C), np.float32)
    for r in res.results:
        acc += np.asarray(r["out"], np.float32)
    return acc.reshape(B, T, C)


# revision 55
# speedup vs baseline: 1.1938x; 1.1891x over previous
"""Distributed GQA attention kernel for 8 TRN2 NeuronCores.

Strategy: tensor-parallel over heads, zero collectives.
Each core d holds 4 query heads + 1 kv head (GQA group d). It computes
q/k/v projections (transposed layouts), RoPE, causal attention, and a
partial o_proj (its heads' contribution to every output element). The
host sums the 8 partial outputs (the "unshard" step).

v2 layout decisions (all aimed at keeping the PE busy):
- Attention runs at 128-query granularity with all 4 local heads packed
  side by side, so score/AV/den matmuls stream 512 columns each.
- RoPE's rotate-half is a DVE stream_shuffle (32-lane group swap); the
  head-dim permutation is chosen so each pair's partner sits 16
  partitions away inside the same 32-partition quadrant.
- o_proj PSUM is double-buffered and its evacuations alternate between
  the Vector and Scalar engines.
- x is loaded 4 contraction-chunks per DMA; the o_proj partial output is
  written with one DMA per 128-token row block.
"""
import sys

sys.path.insert(0, '/opt/trn_rl_repo')

import numpy as np
import ml_dtypes

B, T, C = 2, 2048, 4096
H, KVH, HD = 32, 8, 128
NCORES = 8
N = B * T            # 4096 tokens (batches concatenated)
HL = H // NCORES     # 4 local q heads
TB = 256             # token block for projections
NTB = N // TB        # 16
QB = 128             # query block for attention
KB = 128             # key block
NCH = C // 128       # 32 contraction chunks
SCALE = float(1.0 / np.sqrt(HD))

# Head-dim permutation: pair i=(2i,2i+1) lives in quadrant i//16 at
# offsets i%16 (the "a" half) and 16+i%16 (the "b" half), so rotate-half
# becomes a 16<->16 swap inside each 32-partition stream_shuffle group.
PERM = np.empty(128, np.int64)
for _p in range(128):
    _qd, _r = _p // 32, _p % 32
    _i = _qd * 16 + (_r % 16)
    PERM[_p] = 2 * _i + (1 if _r >= 16 else 0)
IMAP = (np.arange(128) // 32) * 16 + (np.arange(128) % 32) % 16
SSIGN = np.where((np.arange(128) % 32) < 16, 1.0, -1.0).astype(np.float32)
SHUF = [(i + 16) % 32 for i in range(32)]

BF16 = ml_dtypes.bfloat16


def _build(dbg=False):
    import concourse.mybir as mybir
    import concourse.tile as tile
    from concourse import bacc

    dt = mybir.dt
    nc = bacc.Bacc("TRN2", target_bir_lowering=False, debug=False)

    xT_d = nc.declare_dram_parameter("xT", [C, N], dt.bfloat16, isOutput=False)
    wqT_d = nc.declare_dram_parameter("wqT", [C, HL * HD], dt.bfloat16, isOutput=False)
    wkT_d = nc.declare_dram_parameter("wkT", [C, HD], dt.bfloat16, isOutput=False)
    wvT_d = nc.declare_dram_parameter("wvT", [C, HD], dt.bfloat16, isOutput=False)
    woT_d = nc.declare_dram_parameter("woT", [HL * HD, C], dt.bfloat16, isOutput=False)
    cosb_d = nc.declare_dram_parameter("cosb", [128, N], dt.bfloat16, isOutput=False)
    sinb_d = nc.declare_dram_parameter("sinb", [128, N], dt.bfloat16, isOutput=False)
    mask_d = nc.declare_dram_parameter("mask", [128, HL * QB], dt.bfloat16, isOutput=False)
    out_d = nc.declare_dram_parameter("out", [N, C], dt.bfloat16, isOutput=True)

    with tile.TileContext(nc) as tc:
        with (
            tc.tile_pool(name="wts", bufs=1) as wts,
            tc.tile_pool(name="cache", bufs=1) as cache,
            tc.tile_pool(name="xin", bufs=16) as xin,
            tc.tile_pool(name="qk", bufs=2) as qkp,
            tc.tile_pool(name="vt", bufs=2) as vtp,
            tc.tile_pool(name="rope", bufs=5) as ropep,
            tc.tile_pool(name="pt", bufs=8) as ptp,
            tc.tile_pool(name="ps", bufs=2) as psp,
            tc.tile_pool(name="att", bufs=2) as attp,
            tc.tile_pool(name="dn", bufs=2) as dnp,
            tc.tile_pool(name="oev", bufs=3) as oevp,
            tc.tile_pool(name="acc", bufs=2, space="PSUM") as accp,
            tc.tile_pool(name="sps", bufs=2, space="PSUM") as spsp,
            tc.tile_pool(name="avp", bufs=1, space="PSUM") as avpp,
            tc.tile_pool(name="dnp", bufs=1, space="PSUM") as dppp,
            tc.tile_pool(name="ops", bufs=2, space="PSUM") as opsp,
        ):
            # ---------------- resident weights / constants ----------------
            wq_s = wts.tile([128, NCH * HL * 128], dt.bfloat16)   # (c,h) -> col (c*HL+h)*128
            wk_s = wts.tile([128, NCH * 128], dt.bfloat16)
            wv_s = wts.tile([128, NCH * 128], dt.bfloat16)
            wo_s = wts.tile([128, HL * C], dt.bfloat16)           # (h,ct) -> col h*C+ct*512
            cos_s = wts.tile([128, N], dt.bfloat16)
            sin_s = wts.tile([128, N], dt.bfloat16)
            mask_s = wts.tile([128, HL * QB], dt.bfloat16)
            ones_s = wts.tile([128, 128], dt.bfloat16)

            nc.any.memset(ones_s[:, :], 1.0)
            # Startup-critical DMA schedule: the first projection matmuls
            # need x(block0, chunk0) + wq[c0] within a few us, so stream
            # weights in consumption order with exponentially growing
            # pieces on the Sync queue, and push cos/sin/wo (needed tens of
            # us later) to the Activation queue.
            wq_v = wq_s[:, :].rearrange("p (c m) -> p c m", c=NCH)
            wqT_v = wqT_d[:, :].rearrange("(c p) m -> p c m", p=128)
            wo_v = wo_s[:, :].rearrange("p (h m) -> p h m", h=HL)
            woT_v = woT_d[:, :].rearrange("(h p) m -> p h m", p=128)
            wk_v = wk_s[:, :].rearrange("p (c m) -> p c m", c=NCH)
            wkT_v = wkT_d[:, :].rearrange("(c p) m -> p c m", p=128)
            wv_v = wv_s[:, :].rearrange("p (c m) -> p c m", c=NCH)
            wvT_v = wvT_d[:, :].rearrange("(c p) m -> p c m", p=128)

            # only the startup-critical halves go out at t=0; the rest is
            # emitted inside BC(0) (see units_bc) so it doesn't steal DMA
            # bandwidth from the first projection blocks
            nc.scalar.dma_start(cos_s[:, 0:512], cosb_d[:, 0:512])
            nc.scalar.dma_start(sin_s[:, 0:512], sinb_d[:, 0:512])
            nc.scalar.dma_start(wk_v[:, 0:2], wkT_v[:, 0:2])
            nc.scalar.dma_start(wv_v[:, 0:2], wvT_v[:, 0:2])
            nc.scalar.dma_start(wk_v[:, 2:16], wkT_v[:, 2:16])
            nc.scalar.dma_start(wv_v[:, 2:16], wvT_v[:, 2:16])
            nc.scalar.dma_start(wk_v[:, 16:32], wkT_v[:, 16:32])
            nc.scalar.dma_start(wv_v[:, 16:32], wvT_v[:, 16:32])
            for i in range(HL):
                nc.scalar.dma_start(wo_v[:, i, 0:2048], woT_v[:, i, 0:2048])

            xT_v = xT_d[:, :].rearrange("(c p) m -> p c m", p=128)
            pre_xcs = []
            for ci in range(16):
                xc = xin.tile([128, 4 * TB], dt.bfloat16, tag="xc", name="xc")
                pre_xcs.append(xc[:, :].rearrange("p (c m) -> p c m", c=4))

            def wq_piece(c0, c1):
                nc.sync.dma_start(wq_v[:, c0:c1], wqT_v[:, c0:c1])

            def xc_piece(ci, j0, j1):
                # block-0/1 x rides the (idle) gpsimd queue so x and weight
                # issuance run in parallel at startup
                tb0 = ci // 8
                nsl0 = slice(tb0 * TB, (tb0 + 1) * TB)
                nc.gpsimd.dma_start(
                    pre_xcs[ci][:, j0:j1],
                    xT_v[:, (ci % 8) * 4 + j0:(ci % 8) * 4 + j1, nsl0])

            xc_piece(0, 0, 1)
            xc_piece(0, 1, 2)
            xc_piece(0, 2, 4)
            xc_piece(1, 0, 2)
            xc_piece(1, 2, 4)
            for ci in range(2, 16):
                xc_piece(ci, 0, 4)
            wq_piece(0, 1)
            nc.sync.dma_start(mask_s[:, :], mask_d[:, :])
            wq_piece(1, 2)
            wq_piece(2, 4)
            wq_piece(4, 8)
            wq_piece(8, 12)
            wq_piece(12, 16)
            wq_piece(16, 20)
            wq_piece(20, 24)
            wq_piece(24, 28)
            wq_piece(28, 32)

            # per-block k/v cache tiles: exact (uncoarsened) dependency
            # tracking so attention reads never falsely wait on later
            # projection writes. kc: [hd, tok]; vc: [tok%128, ti*128+hd].
            kc = [cache.tile([128, TB], dt.bfloat16, name=f"kc{i}")
                  for i in range(NTB)]
            vc = [cache.tile([128, TB], dt.bfloat16, name=f"vc{i}")
                  for i in range(NTB)]

            deferred_vt = []

            def emit_deferred_vt():
                while deferred_vt:
                    vtb, vtmp = deferred_vt.pop(0)
                    for ti in range(2):
                        nc.sync.dma_start_transpose(
                            vc[vtb][:, ti * 128:(ti + 1) * 128],
                            vtmp[:, ti * 128:(ti + 1) * 128])

            def rope_mul(src, nsl):
                # psum readers first so the accumulator bank frees early
                m1 = ropep.tile([128, TB], dt.float32, tag="m1")
                nc.vector.tensor_mul(m1[:, :], src, cos_s[:, nsl])
                u = ropep.tile([128, TB], dt.float32, tag="u")
                nc.vector.tensor_mul(u[:, :], src, sin_s[:, nsl])
                return m1, u

            def rope_fin(dst, m1, u):
                # dst = m1 + shuffle16(u)
                sw = ropep.tile([128, TB], dt.float32, tag="sw")
                nc.vector.stream_shuffle(sw[:, :], u[:, :], SHUF)
                nc.vector.tensor_add(dst, m1[:, :], sw[:, :])

            def units_a(tb, ready_xcs=None):
                """Closure list for the projection phase of block tb.
                Returns (units, q_qv). Units must be called in order."""
                nsl = slice(tb * TB, (tb + 1) * TB)
                st_ = {}

                def u_start():
                    if ready_xcs is not None:
                        st_['xcs'] = ready_xcs
                        if tb > 0:
                            emit_deferred_vt()
                    else:
                        xcs = []
                        for ci in range(8):
                            xc = xin.tile([128, 4 * TB], dt.bfloat16,
                                          tag="xc", name="xc")
                            xc_v = xc[:, :].rearrange("p (c m) -> p c m", c=4)
                            nc.sync.dma_start(
                                xc_v[:, :, :],
                                xT_d[:, nsl].rearrange("(c p) m -> p c m", p=128)[
                                    :, ci * 4:(ci + 1) * 4])
                            xcs.append(xc_v)
                        st_['xcs'] = xcs
                        emit_deferred_vt()
                    st_['t0'] = accp.tile([128, 512], dt.float32, tag="acc",
                                          name="t0")
                    st_['t2'] = accp.tile([128, 512], dt.float32, tag="acc",
                                          name="t2")

                def u_p1(c0):
                    t0, t2, xcs = st_['t0'], st_['t2'], st_['xcs']
                    for c in range(c0, c0 + 2):
                        xc = xcs[c // 4][:, c % 4, :]
                        st = c == 0
                        sp = c == NCH - 1
                        for h in range(2):
                            nc.tensor.matmul(
                                t0[:, h * 256:(h + 1) * 256],
                                wq_s[:, (c * HL + h) * 128:(c * HL + h + 1) * 128],
                                xc, start=st and h == 0, stop=sp)
                        nc.tensor.matmul(
                            t2[:, 0:256],
                            wk_s[:, c * 128:(c + 1) * 128], xc,
                            start=st, stop=sp)
                        # v in k-style ([hd, tok]); transposed to cache
                        # layout later via DMA transpose
                        nc.tensor.matmul(
                            t2[:, 256:512],
                            wv_s[:, c * 128:(c + 1) * 128], xc,
                            start=False, stop=sp)

                def u_rope1():
                    t0, t2 = st_['t0'], st_['t2']
                    mq0 = rope_mul(t0[:, 0:256], nsl)
                    mq1 = rope_mul(t0[:, 256:512], nsl)      # frees t0
                    mk = rope_mul(t2[:, 0:256], nsl)
                    vtmp = vtp.tile([128, TB], dt.bfloat16, tag="vtmp",
                                    name="vtmp")
                    nc.vector.tensor_copy(vtmp[:, :], t2[:, 256:512])
                    # transposes into the v cache are DEFERRED: emitted on
                    # the Sync queue only after the next block's x-prefetch
                    # DMAs, by which time vtmp is ready (no queue blocking)
                    deferred_vt.append((tb, vtmp))
                    rope_fin(q_qv[:, 0, :], *mq0)
                    rope_fin(q_qv[:, 1, :], *mq1)
                    rope_fin(kc[tb][:, :], *mk)
                    st_['t1'] = accp.tile([128, 512], dt.float32, tag="acc",
                                          name="t1")

                def u_p2(c0):
                    t1, xcs = st_['t1'], st_['xcs']
                    for c in range(c0, c0 + 2):
                        xc = xcs[c // 4][:, c % 4, :]
                        sp = c == NCH - 1
                        for h in range(2):
                            nc.tensor.matmul(
                                t1[:, h * 256:(h + 1) * 256],
                                wq_s[:, (c * HL + h + 2) * 128:(c * HL + h + 3) * 128],
                                xc, start=c == 0 and h == 0, stop=sp)

                def u_rope2():
                    t1 = st_['t1']
                    mq2 = rope_mul(t1[:, 0:256], nsl)
                    mq3 = rope_mul(t1[:, 256:512], nsl)      # frees t1
                    rope_fin(q_qv[:, 2, :], *mq2)
                    rope_fin(q_qv[:, 3, :], *mq3)

                q_sb = qkp.tile([128, HL * TB], dt.bfloat16, tag="qsb",
                                name="q_sb")
                q_qv = q_sb[:, :].rearrange("p (h q) -> p h q", h=HL)
                units = [u_start]
                units += [(lambda c0=c0: u_p1(c0)) for c0 in range(0, NCH, 2)]
                units.append(u_rope1)
                units += [(lambda c0=c0: u_p2(c0)) for c0 in range(0, NCH, 2)]
                units.append(u_rope2)
                return units, q_qv

            def units_bc(tb, q_qv):
                """Closure list for attention + o_proj of block tb."""
                b = tb // 8
                units = []
                st_ = {}
                for qh in range(2):
                    qbl = (tb % 8) * 2 + qh        # in-batch 128-query block
                    nkb = qbl + 1
                    qrhs = q_qv[:, :, qh * 128:(qh + 1) * 128]

                    def u_binit(qh=qh):
                        if tb == 0 and qh == 0:
                            # deferred bulk loads: needed from ~45us on
                            for i in range(HL):
                                nc.sync.dma_start(wo_v[:, i, 2048:C],
                                                  woT_v[:, i, 2048:C])
                        if tb == 1 and qh == 0:
                            # cos/sin bulk: first consumer is RoPE(tb2)
                            nc.sync.dma_start(cos_s[:, 512:N],
                                              cosb_d[:, 512:N])
                            nc.sync.dma_start(sin_s[:, 512:N],
                                              sinb_d[:, 512:N])
                        st_[qh, 'at4'] = avpp.tile([128, 512], dt.float32,
                                                   tag="at4", name="at4")
                        st_[qh, 'den'] = dppp.tile([128, 512], dt.float32,
                                                   tag="den", name="den")
                    units.append(u_binit)

                    def u_kbl(kbl, qh=qh, nkb=nkb, qrhs=qrhs):
                        at4, den = st_[qh, 'at4'], st_[qh, 'den']
                        kbg = b * 16 + kbl
                        sT = spsp.tile([128, 512], dt.float32, tag="sT",
                                       name="sT")
                        nc.tensor.matmul(
                            sT[:, :], kc[kbg // 2][:, (kbg % 2) * 128:(kbg % 2 + 1) * 128],
                            qrhs, start=True, stop=True)
                        pT = ptp.tile([128, 512], dt.bfloat16, tag="pT",
                                      name="pT")
                        nc.scalar.activation(pT[:, :], sT[:, :],
                                             mybir.ActivationFunctionType.Exp,
                                             scale=SCALE)
                        if kbl == nkb - 1:
                            nc.vector.tensor_mul(pT[:, :], pT[:, :], mask_s[:, :])
                        nc.tensor.matmul(
                            at4[:, :],
                            vc[kbg // 2][:, (kbg % 2) * 128:(kbg % 2 + 1) * 128],
                            pT[:, :], start=kbl == 0, stop=kbl == nkb - 1)
                        # denominator: fold up to 4 pT tiles with cheap bf16
                        # DVE adds so only every 4th kbl pays a ones-matmul
                        grp = st_.setdefault((qh, 'grp'), [])
                        grp.append(pT)
                        if len(grp) == 4 or kbl == nkb - 1:
                            acc = grp[0]
                            for nxt in grp[1:]:
                                ps = psp.tile([128, 512], dt.bfloat16,
                                              tag="ps", name="ps")
                                nc.vector.tensor_add(ps[:, :], acc[:, :],
                                                     nxt[:, :])
                                acc = ps
                            nc.tensor.matmul(den[:, :], ones_s[:, :],
                                             acc[:, :], start=kbl < 4,
                                             stop=kbl == nkb - 1)
                            grp.clear()
                    units += [(lambda kbl=kbl, f=u_kbl: f(kbl))
                              for kbl in range(nkb)]

                    def u_soft(qh=qh):
                        at4, den = st_[qh, 'at4'], st_[qh, 'den']
                        denb = dnp.tile([128, 512], dt.float32, tag="denb",
                                        name="denb")
                        nc.vector.reciprocal_approx_fast(denb[:, :], den[:, :])
                        attn = attp.tile([128, 512], dt.bfloat16, tag="attn",
                                         name="attn")
                        nc.vector.tensor_mul(attn[:, :], at4[:, :], denb[:, :])
                        st_[qh, 'attn'] = attn
                        st_[qh, 'oev'] = oevp.tile([128, C], dt.bfloat16,
                                                   tag="oev", name="oev")
                    units.append(u_soft)

                    def u_oproj(ct, qh=qh):
                        attn, oev = st_[qh, 'attn'], st_[qh, 'oev']
                        ops = opsp.tile([128, 512], dt.float32, tag="ops",
                                        name="ops")
                        for h in range(HL):
                            nc.tensor.matmul(
                                ops[:, :],
                                attn[:, h * 128:(h + 1) * 128],
                                wo_s[:, h * C + ct * 512:h * C + (ct + 1) * 512],
                                start=(h == 0), stop=(h == HL - 1))
                        osl = slice(ct * 512, (ct + 1) * 512)
                        if ct % 2 == 0:
                            nc.vector.tensor_copy(oev[:, osl], ops[:, :])
                        else:
                            nc.scalar.activation(
                                oev[:, osl], ops[:, :],
                                mybir.ActivationFunctionType.Copy)
                        if tb >= NTB - 2 and ct % 2 == 1:
                            # last blocks: drain the output in quarters so
                            # the final DMA isn't serialized after compute
                            r0 = tb * TB + qh * 128
                            nc.sync.dma_start(
                                out_d[r0:r0 + 128, (ct - 1) * 512:(ct + 1) * 512],
                                oev[:, (ct - 1) * 512:(ct + 1) * 512])
                    units += [(lambda ct=ct, f=u_oproj: f(ct))
                              for ct in range(C // 512)]

                    def u_dma(qh=qh):
                        if tb >= NTB - 2:
                            return
                        r0 = tb * TB + qh * 128
                        nc.gpsimd.dma_start(out_d[r0:r0 + 128, :],
                                            st_[qh, 'oev'][:, :])
                    units.append(u_dma)
                return units

            def run_merged(ua, ub):
                # proportional stable merge of the two unit lists
                seq = [((i + 0.5) / len(ua), 0, i, u) for i, u in enumerate(ua)]
                seq += [((i + 0.5) / len(ub), 1, i, u) for i, u in enumerate(ub)]
                for _, _, _, u in sorted(seq, key=lambda t: (t[0], t[1])):
                    u()

            # software pipeline: interleave the emission of A(tb) with
            # B/C(tb-1) so both phases' work is pending at every point and
            # the scheduler can fill any cross-engine stall.
            prev = None
            for tb in range(NTB):
                rx = pre_xcs[8 * tb:8 * tb + 8] if tb < 2 else None
                ua, q_qv = units_a(tb, ready_xcs=rx)
                if prev is None:
                    for u in ua:
                        u()
                else:
                    run_merged(ua, units_bc(tb - 1, prev))
                prev = q_qv
            emit_deferred_vt()
            for u in units_bc(NTB - 1, prev):
                u()
    nc.finalize()
    return nc


def _prep_shared(x, freqs_cis):
    xf = np.asarray(x, np.float32).reshape(N, C)
    xT = np.ascontiguousarray(xf.T).astype(BF16)
    fc = np.asarray(freqs_cis, np.float32)
    cos = np.ascontiguousarray(fc[:, :, 0].T)   # [64, T]
    sin = np.ascontiguousarray(fc[:, :, 1].T)
    cosb = np.ascontiguousarray(np.tile(cos[IMAP], (1, B))).astype(BF16)
    sinb = np.ascontiguousarray(
        np.tile(sin[IMAP] * SSIGN[:, None], (1, B))).astype(BF16)
    # diagonal-block causal mask [key, (head, query)]: key <= query
    m = np.triu(np.ones((KB, QB), np.float32)).astype(BF16)
    mask = np.ascontiguousarray(np.tile(m, (1, HL)))
    return xT, cosb, sinb, mask


def _prep_core(d, wq_p, wk_p, wv_f, wo_f):
    qsl = slice(d * HL * HD, (d + 1) * HL * HD)
    ksl = slice(d * HD, (d + 1) * HD)
    wqT = np.ascontiguousarray(wq_p[qsl].T).astype(BF16)
    wkT = np.ascontiguousarray(wk_p[ksl].T).astype(BF16)
    wvT = np.ascontiguousarray(wv_f[ksl].T).astype(BF16)
    woT = np.ascontiguousarray(wo_f[:, qsl].T).astype(BF16)
    return wqT, wkT, wvT, woT


_NC_CACHE = []


def kernel(x, freqs_cis, wq, wk, wv, wo):
    from concourse import bass_utils

    if not _NC_CACHE:
        _NC_CACHE.append(_build())
    nc = _NC_CACHE[0]

    xT, cosb, sinb, mask = _prep_shared(x, freqs_cis)
    wq_p = np.asarray(wq, np.float32).reshape(H, HD, C)[:, PERM, :].reshape(H * HD, C)
    wk_p = np.asarray(wk, np.float32).reshape(KVH, HD, C)[:, PERM, :].reshape(KVH * HD, C)
    wv_f = np.asarray(wv, np.float32)
    wo_f = np.asarray(wo, np.float32)

    in_maps = []
    for d in range(NCORES):
        wqT, wkT, wvT, woT = _prep_core(d, wq_p, wk_p, wv_f, wo_f)
        in_maps.append({
            "xT": xT, "wqT": wqT, "wkT": wkT, "wvT": wvT, "woT": woT,
            "cosb": cosb, "sinb": sinb, "mask": mask,
        })
    res = bass_utils.run_bass_kernel_spmd(nc, in_maps, core_ids=list(range(NCORES)))
    acc = np.zeros((N, C), np.float32)
    for r in res.results:
        acc += np.asarray(r["out"], np.float32)
    return acc.reshape(B, T, C)
